# revision 11
# baseline (speedup 1.0000x reference)
"""Trainium2 Bass kernel for nn_MultiHeadMLPAttentionModel.

Model: per (b, n) point: pairwise = [radar_b(4), pt(2)] (radar constant over n).
  h1 = relu(pairwise @ enc_w1 + enc_b1)            [B,N,64]
  pf = h1 @ enc_w2 + enc_b2                        [B,N,64]
  sh = relu(einsum('bnf,hfd', pairwise, sc_w1) + sc_b1)
  logits = einsum('bnhd,hd', sh, sc_w2) + sc_b2    [B,N,4]
  w = softmax(logits, axis=n)
  ctx = einsum('bnh,bnd', w, pf)  -> out MLP -> [B]

Key algebraic restructurings used here:
  * pooling commutes with the (linear) second encoder layer since softmax
    weights sum to 1:  ctx = (sum_n w * h1) @ enc_w2 + enc_b2.  This removes
    the N-scale enc2 matmul entirely.
  * sc_b2 is constant over n, so it drops out of the softmax.
  * the radar part of pairwise is constant over n, so all layer-1 radar
    contributions fold into per-b bias vectors (computed on host: ~200 KFLOP
    of the model's 13 GFLOP).
  * softmax is computed without max-subtraction (logits are O(1) for this
    model; exp is evaluated in fp32) and normalization is deferred: the
    pooling matmul accumulates unnormalized sum_n exp(l)*h1 plus sum_n exp(l)
    (via an appended ones column), and the division happens once per b.

Sharding: pure data parallel over B: 8 cores x 16 rows each.  One SPMD Bass
program; per-core inputs differ only in data.
"""

import numpy as np

import concourse.bass as bass
import concourse.tile as tile
from concourse import bacc, mybir

B, N, HID, HEADS = 128, 8192, 64, 4
NCORES = 8
BPC = B // NCORES  # 16 batch rows per core
CHUNK = 512
NCH = N // CHUNK  # 16
NB = N // 128  # 64 point-blocks of 128

F32 = mybir.dt.float32
BF16 = mybir.dt.bfloat16
AF = mybir.ActivationFunctionType
ALU = mybir.AluOpType


def build_nc(reps=1, phases="ATPD"):
    from contextlib import ExitStack

    nc = bacc.Bacc()
    f32 = F32

    xp_d = nc.dram_tensor("xp", [BPC, 6, N], BF16, kind="ExternalInput")
    xpa_d = nc.dram_tensor("xpa", [NCH, 4, BPC * CHUNK], BF16, kind="ExternalInput")
    # replicated at 4 partition-group offsets for PE row-tiling
    cb1_d = nc.dram_tensor("cb1", [128, BPC], f32, kind="ExternalInput")
    cb2_d = nc.dram_tensor("cb2", [128, BPC], f32, kind="ExternalInput")
    wp_d = nc.dram_tensor("wp", [4, 256], BF16, kind="ExternalInput")
    w2a_d = nc.dram_tensor("w2a", [128, BPC * 32], BF16, kind="ExternalInput")
    w2b_d = nc.dram_tensor("w2b", [128, BPC * 32], BF16, kind="ExternalInput")
    wenm_d = nc.dram_tensor("wenm", [6, BPC * 65], BF16, kind="ExternalInput")
    ew2b_d = nc.dram_tensor("ew2b", [65, 64], f32, kind="ExternalInput")
    ow1_d = nc.dram_tensor("ow1", [64, 256], f32, kind="ExternalInput")
    ob1_d = nc.dram_tensor("ob1", [1, 64], f32, kind="ExternalInput")
    w2o_d = nc.dram_tensor("w2o", [65, 1], f32, kind="ExternalInput")
    id128_d = nc.dram_tensor("id128", [128, 128], BF16, kind="ExternalInput")
    on16_d = nc.dram_tensor("on16", [1, BPC], f32, kind="ExternalInput")
    out_d = nc.dram_tensor("out", [BPC], f32, kind="ExternalOutput")

    with tile.TileContext(nc) as tc, ExitStack() as ctx:
        consts = ctx.enter_context(tc.tile_pool(name="consts", bufs=1))

        def cload(dram, shape, nm, dt=f32):
            t = consts.tile(shape, dt, name=nm, tag=nm)
            nc.sync.dma_start(t[:], dram[:])
            return t

        # wp replicated at partition offsets {0,32,64,96} so the four K=4
        # score matmuls of a b-pair can run concurrently in distinct PE
        # row-groups (tile_position row tiling).
        wp_s = consts.tile([100, 256], BF16, name="wp_s", tag="wp_s")
        for u in range(4):
            nc.sync.dma_start(wp_s[32 * u : 32 * u + 4, :], wp_d[:])
        cb1_s = cload(cb1_d, [128, BPC], "cb1_s")
        cb2_s = cload(cb2_d, [128, BPC], "cb2_s")
        w2a_s = cload(w2a_d, [128, BPC * 32], "w2a_s", BF16)
        w2b_s = cload(w2b_d, [128, BPC * 32], "w2b_s", BF16)
        ew2b_s = cload(ew2b_d, [65, 64], "ew2b_s")
        ow1_s = cload(ow1_d, [64, 256], "ow1_s")
        ob1_s = cload(ob1_d, [1, 64], "ob1_s")
        w2o_s = cload(w2o_d, [65, 1], "w2o_s")
        id128_s = cload(id128_d, [128, 128], "id128_s", BF16)
        on16_s = cload(on16_d, [1, BPC], "on16_s")
        # wenm replicated at partition offsets {0,32} for 2-way row-tiled
        # encoder matmuls
        wenm_s = consts.tile([38, BPC * 65], BF16, name="wenm_s", tag="wenm_s")
        for r in range(2):
            nc.sync.dma_start(wenm_s[32 * r : 32 * r + 6, :], wenm_d[:])

        # n-major exp(logits): block t occupies cols [t*128, (t+1)*128);
        # within a block: partition p = n offset, col = logits row r where
        # r = 64*(b%2) + 32*(h//2) + 2*(b//2) + (h%2)
        enm = consts.tile([128, NB * 128], BF16, name="enm", tag="enm")
        ctxnT = consts.tile([65, 64], f32, name="ctxnT", tag="ctxnT")
        obuf = consts.tile([65, BPC], f32, name="obuf", tag="obuf")
        fct = consts.tile([64, 64], f32, name="fct", tag="fct")
        res = consts.tile([1, BPC], f32, name="res", tag="res")
        ones64 = consts.tile([1, 64], f32, name="ones64", tag="ones64")
        rz64 = consts.tile([1, 64], f32, name="rz64", tag="rz64")
        rbc_sb = consts.tile([64, 64], f32, name="rbc_sb", tag="rbc_sb")
        nc.vector.memset(obuf[64:65, :], 1.0)
        nc.vector.memset(ones64[:], 1.0)

        if "A" not in phases:
            nc.vector.memset(enm[:, 0:8], 0.0)
        for _rep in range(reps):
            _build_body(
                nc, tc, xp_d, xpa_d, out_d,
                wp_s, cb1_s, cb2_s, w2a_s, w2b_s, wenm_s, ew2b_s, ow1_s,
                ob1_s, w2o_s, id128_s, on16_s,
                enm, ctxnT, obuf, fct, res, ones64, rz64, rbc_sb, phases,
            )

    if not nc.is_finalized():
        nc.finalize()
    return nc


def _build_body(
    nc, tc, xp_d, xpa_d, out_d,
    wp_s, cb1_s, cb2_s, w2a_s, w2b_s, wenm_s, ew2b_s, ow1_s,
    ob1_s, w2o_s, id128_s, on16_s,
    enm, ctxnT, obuf, fct, res, ones64, rz64, rbc_sb, phases="ATPD",
):
    from contextlib import ExitStack

    f32 = F32
    if "A" in phases:
        # ---- Phase A: score-net hidden + logits (feature-major) ----------
        with ExitStack() as pctx:
            xpool = pctx.enter_context(tc.tile_pool(name="xpA", bufs=3))
            shpool = pctx.enter_context(tc.tile_pool(name="shp", bufs=8))
            epool = pctx.enter_context(tc.tile_pool(name="ep", bufs=2))
            psA = pctx.enter_context(tc.tile_pool(name="psA", bufs=4, space="PSUM"))
            psL = pctx.enter_context(tc.tile_pool(name="psL", bufs=2, space="PSUM"))
            psT = pctx.enter_context(tc.tile_pool(name="psT", bufs=2, space="PSUM"))

            xpcs = {}

            def load_xpc(c):
                # point data replicated at 4 partition-group offsets for
                # row-tiled matmuls
                t = xpool.tile([100, BPC * CHUNK], BF16, name="xpc", tag="xpc")
                for u in range(4):
                    nc.sync.dma_start(t[32 * u : 32 * u + 4, :], xpa_d[c])
                xpcs[c] = t

            DEPTH = 2  # software-pipeline depth (in b-pairs)
            lg_done = {}

            def expose(c):
                # exp of chunk c's logits, then transpose its 4 blocks n-major
                lg = lg_done.pop(c)
                e_c = epool.tile([128, CHUNK], BF16, name="e_c", tag="e_c")
                nc.scalar.activation(e_c[:], lg[:], AF.Exp)
                for j in range(CHUNK // 128):
                    t = c * (CHUNK // 128) + j
                    t_ps = psT.tile([128, 128], BF16, name="t_ps", tag="tp")
                    nc.tensor.transpose(
                        t_ps[:], e_c[:, j * 128 : (j + 1) * 128], id128_s[:]
                    )
                    nc.vector.tensor_copy(
                        out=enm[:, t * 128 : (t + 1) * 128], in_=t_ps[:]
                    )

            load_xpc(0)
            if NCH > 1:
                load_xpc(1)
            for c in range(NCH):
                if c + 2 < NCH:
                    load_xpc(c + 2)
                if c > 0:
                    expose(c - 1)
                xpc = xpcs.pop(c)
                lg_ps = psL.tile([128, CHUNK], f32, name="lg_ps", tag="lg")
                pend = []

                def drain_lg(lg_ps=lg_ps):
                    # logits for a b-pair: four K=128,M=32 matmuls run
                    # concurrently in four PE column-groups, each accumulating
                    # its own 32-partition slice of lg_ps over the 8 pairs
                    j, sbs = pend.pop(0)
                    for u in range(4):
                        w2 = w2a_s if u % 2 == 0 else w2b_s
                        b = 2 * j + u // 2
                        nc.tensor.matmul(
                            lg_ps[32 * u : 32 * u + 32, :],
                            w2[:, b * 32 : (b + 1) * 32],
                            sbs[u][:],
                            start=(j == 0),
                            stop=(j == BPC // 2 - 1),
                            tile_position=(0, 32 * u),
                            skip_group_check=True,
                        )

                for j in range(BPC // 2):
                    b0 = 2 * j
                    # four K=4 score matmuls (two b's x two head-pairs) run
                    # concurrently in four PE row-groups
                    sh_ps, sh_sb = [], []
                    for u in range(4):
                        b = b0 + u // 2
                        xb = xpc[
                            32 * u : 32 * u + 4, b * CHUNK : (b + 1) * CHUNK
                        ]
                        wslice = wp_s[
                            32 * u : 32 * u + 4,
                            (u % 2) * 128 : (u % 2) * 128 + 128,
                        ]
                        ps = psA.tile([128, CHUNK], f32, name="sh_ps", tag="sh")
                        nc.tensor.matmul(
                            ps[:], wslice, xb, start=True, stop=True,
                            tile_position=(32 * u, 0),
                        )
                        sh_ps.append(ps)
                    for u in range(4):
                        b = b0 + u // 2
                        cb = cb1_s if u % 2 == 0 else cb2_s
                        sb = shpool.tile([128, CHUNK], BF16, name="sh_sb", tag="shs")
                        if u % 2 == j % 2:
                            nc.scalar.activation(
                                sb[:], sh_ps[u][:], AF.Relu, bias=cb[:, b : b + 1]
                            )
                        else:
                            nc.vector.tensor_scalar(
                                sb[:], sh_ps[u][:], cb[:, b : b + 1], 0.0,
                                ALU.add, ALU.max,
                            )
                        sh_sb.append(sb)
                    # drain order within a pair: (s1 b0, s2 b0, s1 b1, s2 b1)
                    pend.append((j, sh_sb))
                    while len(pend) > DEPTH:
                        drain_lg()
                while pend:
                    drain_lg()
                lg_done[c] = lg_ps
            expose(NCH - 1)

    if "P" in phases:
        # ---- Phase P: n-major encoder hidden + weighted pooling ----------
        # logits row layout inside an enm block (128 cols):
        #   r = 64*(b%2) + 32*(h//2) + 2*(b//2) + (h%2)
        enm_r = enm.rearrange(
            "p (t par hh bb) -> p t par hh bb", par=2, hh=2, bb=32
        )
        with ExitStack() as pctx:
            xbpool = pctx.enter_context(tc.tile_pool(name="xpC", bufs=2))
            h1pool = pctx.enter_context(tc.tile_pool(name="h1p", bufs=3))
            psH = pctx.enter_context(tc.tile_pool(name="psH", bufs=2, space="PSUM"))
            psC = pctx.enter_context(tc.tile_pool(name="psC", bufs=2, space="PSUM"))
            GB = 8  # blocks per h1/pool group (2 row-tiles x 4)
            xpbs = {}

            def load_xpb(b):
                # point rows replicated at partition offsets {0,32} for 2-way
                # row-tiled encoder matmuls
                t = xbpool.tile([38, N], BF16, name="xpb", tag="xpb")
                for r in range(2):
                    nc.sync.dma_start(t[32 * r : 32 * r + 6, :], xp_d[b])
                xpbs[b] = t

            load_xpb(0)
            for b in range(BPC):
                if b + 1 < BPC:
                    load_xpb(b + 1)
                xpb = xpbs.pop(b)
                c1_ps = psC.tile([65, 4], f32, name="c1_ps", tag="c1")
                hpend = []

                def drain_pool(c1_ps=c1_ps, b=b):
                    # pooling: stationary = h1 block, moving = 4 exp columns
                    g, h1_sb = hpend.pop(0)
                    for k in range(GB):
                        t = g * GB + k
                        nc.tensor.matmul(
                            c1_ps[:],
                            h1_sb[:, k * 65 : (k + 1) * 65],
                            enm_r[
                                :, t, b % 2, :,
                                2 * (b // 2) : 2 * (b // 2) + 2,
                            ],
                            start=(t == 0),
                            stop=(t == NB - 1),
                            skip_group_check=True,
                        )

                for g in range(NB // GB):
                    h1_ps = [
                        psH.tile([128, 4 * 65], f32, name="h1_ps", tag="h1")
                        for _ in range(2)
                    ]
                    for r in range(2):
                        for jj in range(4):
                            t = g * GB + r * 4 + jj
                            nc.tensor.matmul(
                                h1_ps[r][:, jj * 65 : (jj + 1) * 65],
                                xpb[32 * r : 32 * r + 6, t * 128 : (t + 1) * 128],
                                wenm_s[32 * r : 32 * r + 6, b * 65 : (b + 1) * 65],
                                start=True,
                                stop=True,
                                tile_position=(32 * r, 0),
                                skip_group_check=True,
                            )
                    h1_sb = h1pool.tile([128, GB * 65], BF16, name="h1_sb", tag="h1s")
                    for r in range(2):
                        dst = h1_sb[:, r * 260 : (r + 1) * 260]
                        if r == g % 2:
                            nc.vector.tensor_scalar(
                                dst, h1_ps[r][:], 0.0, None, ALU.max
                            )
                        else:
                            nc.scalar.activation(dst, h1_ps[r][:], AF.Relu)
                    hpend.append((g, h1_sb))
                    if len(hpend) > 1:
                        drain_pool()
                while hpend:
                    drain_pool()
                # c1_ps rows 0:64 = unnormalized context (hidden-major), row
                # 64 = sum of exp; normalization deferred to phase D
                nc.vector.tensor_copy(
                    out=ctxnT[:, b * 4 : (b + 1) * 4], in_=c1_ps[:]
                )

    if "D" in phases:
        # ---- Phase D: pooled-context encoder layer 2 + output MLP --------
        with ExitStack() as pctx:
            psD = pctx.enter_context(tc.tile_pool(name="psD", bufs=1, space="PSUM"))
            # fct_un[:, 4b+h] = sum_e * (enc_w2.T ctx_norm + enc_b2)
            fct_ps = psD.tile([64, 64], f32, name="fct_ps", tag="fctp")
            nc.tensor.matmul(fct_ps[:], ew2b_s[:], ctxnT[:], start=True, stop=True)
            # normalize columns by 1/sum_e via a rank-1 broadcast matmul
            nc.vector.reciprocal(rz64[:], ctxnT[64:65, :])
            rbc_ps = psD.tile([64, 64], f32, name="rbc_ps", tag="rbcp")
            nc.tensor.matmul(rbc_ps[:], ones64[:], rz64[:], start=True, stop=True)
            nc.vector.tensor_copy(out=rbc_sb[:], in_=rbc_ps[:])
            nc.vector.scalar_tensor_tensor(
                fct[:], fct_ps[:], 1.0, rbc_sb[:], ALU.mult, ALU.mult
            )
            fct_bh = fct.rearrange("d (b h) -> d b h", h=HEADS)
            o1_ps = psD.tile([64, BPC], f32, name="o1_ps", tag="o1p")
            for h in range(HEADS):
                nc.tensor.matmul(
                    o1_ps[:],
                    ow1_s[:, h * 64 : (h + 1) * 64],
                    fct_bh[:, :, h],
                    start=(h == 0),
                    stop=False,
                    skip_group_check=True,
                )
            nc.tensor.matmul(
                o1_ps[:], ob1_s[:], on16_s[:], start=False, stop=True,
                skip_group_check=True,
            )
            nc.scalar.activation(obuf[0:64, :], o1_ps[:], AF.Relu)
            fin_ps = psD.tile([1, BPC], f32, name="fin_ps", tag="finp")
            nc.tensor.matmul(fin_ps[:], w2o_s[:], obuf[:], start=True, stop=True)
            nc.vector.tensor_copy(out=res[:], in_=fin_ps[:])
            nc.sync.dma_start(out_d.rearrange("(a n) -> a n", a=1), res[:])


def make_in_maps(inputs):
    """Host-side marshalling: slice B across cores and pack weights into the
    layouts the device program expects.

    bf16 note: the big streamed matmuls run in bf16.  To avoid systematic
    model-weight rounding, layer-1 weights are split hi/lo across extra
    contraction rows (w = hi + lo with both bf16); per-point input rounding
    is stochastic and averages out in the softmax pooling."""
    import ml_dtypes

    bf = ml_dtypes.bfloat16
    f = np.float32

    def split(a):
        hi = a.astype(bf)
        lo = (a - hi.astype(f)).astype(bf)
        return hi, lo
    radar = np.concatenate(
        [np.asarray(inputs["radar_xy"], f), np.asarray(inputs["radar_dir"], f)], axis=1
    )  # [B, 4]
    pts = np.asarray(inputs["pts"], f)
    enc_w1 = np.asarray(inputs["enc_w1"], f)
    enc_b1 = np.asarray(inputs["enc_b1"], f)
    enc_w2 = np.asarray(inputs["enc_w2"], f)
    enc_b2 = np.asarray(inputs["enc_b2"], f)
    sc_w1 = np.asarray(inputs["sc_w1"], f)
    sc_b1 = np.asarray(inputs["sc_b1"], f)
    sc_w2 = np.asarray(inputs["sc_w2"], f)
    out_w1 = np.asarray(inputs["out_w1"], f)
    out_b1 = np.asarray(inputs["out_b1"], f)
    out_w2 = np.asarray(inputs["out_w2"], f)
    out_b2 = np.asarray(inputs["out_b2"], f)

    # per-b layer-1 bias vectors (radar is constant over n)
    cb_sc = np.einsum("br,hrd->bhd", radar, sc_w1[:, :4, :]) + sc_b1  # [B, 4, 64]
    cb_enc = radar @ enc_w1[:4] + enc_b1  # [B, 64]

    # xp rows: [xh, yh, xh, yh, 1, 1] (bf16); rows 0-3 feed the weight-split
    # layer-1 matmuls, rows 4-5 carry the (split) bias contraction.
    xp = np.empty((B, 6, N), bf)
    xh = pts[:, :, 0].astype(bf)
    yh = pts[:, :, 1].astype(bf)
    xp[:, 0] = xh
    xp[:, 1] = yh
    xp[:, 2] = xh
    xp[:, 3] = yh
    xp[:, 4] = 1.0
    xp[:, 5] = 1.0

    # wp rows: [wxh, wyh, wxl, wyl] against xp rows [xh, yh, xh, yh]
    wp = np.empty((4, 256), bf)
    for h in range(HEADS):
        wxh, wxl = split(sc_w1[h, 4, :])
        wyh, wyl = split(sc_w1[h, 5, :])
        wp[0, h * 64 : (h + 1) * 64] = wxh
        wp[1, h * 64 : (h + 1) * 64] = wyh
        wp[2, h * 64 : (h + 1) * 64] = wxl
        wp[3, h * 64 : (h + 1) * 64] = wyl
    # heads 0,1 feed sh1 (wp cols 0:128), heads 2,3 feed sh2 (cols 128:256)

    # per-b logits stationaries [128, 32]: col = 2*(b//2) + (h%2), rows
    # h%2 * 64.  w2a carries heads {0,1} (for the s1 tiles), w2b heads {2,3}.
    w2a = np.zeros((128, BPC * 32), bf)
    w2b = np.zeros((128, BPC * 32), bf)
    for bl in range(BPC):
        loc = bl * 32 + 2 * (bl // 2)
        w2a[0:64, loc + 0] = sc_w2[0]
        w2a[64:128, loc + 1] = sc_w2[1]
        w2b[0:64, loc + 0] = sc_w2[2]
        w2b[64:128, loc + 1] = sc_w2[3]

    ew2b = np.concatenate([enc_w2, enc_b2[None, :]], axis=0)  # [65, 64]
    ow1 = np.empty((64, 256), f)
    for h in range(HEADS):
        ow1[:, h * 64 : (h + 1) * 64] = out_w1[h * 64 : (h + 1) * 64, :]
    ob1 = np.ascontiguousarray(out_b1[None, :])
    w2o = np.concatenate([out_w2, out_b2[None, :]], axis=0)  # [65, 1]
    id128 = np.eye(128, dtype=bf)
    on16 = np.ones((1, BPC), f)

    in_maps = []
    for c in range(NCORES):
        sl = slice(c * BPC, (c + 1) * BPC)
        cb1 = np.ascontiguousarray(cb_sc[sl, 0:2].reshape(BPC, 128).T)
        cb2 = np.ascontiguousarray(cb_sc[sl, 2:4].reshape(BPC, 128).T)
        # wenm rows [wxh, wyh, wxl, wyl, bh, bl] vs xp rows [xh, yh, xh, yh, 1, 1]
        wenm = np.zeros((6, BPC * 65), bf)
        exh, exl = split(enc_w1[4])
        eyh, eyl = split(enc_w1[5])
        for bl in range(BPC):
            s = slice(bl * 65, bl * 65 + 64)
            wenm[0, s] = exh
            wenm[1, s] = eyh
            wenm[2, s] = exl
            wenm[3, s] = eyl
            bh, blo = split(cb_enc[c * BPC + bl])
            wenm[4, s] = bh
            wenm[5, s] = blo
            wenm[4, bl * 65 + 64] = 1.0
        xpc_core = np.ascontiguousarray(xp[sl])
        xpa = np.ascontiguousarray(
            xpc_core[:, 0:4]
            .reshape(BPC, 4, NCH, CHUNK)
            .transpose(2, 1, 0, 3)
            .reshape(NCH, 4, BPC * CHUNK)
        )
        in_maps.append(
            dict(
                xp=xpc_core,
                xpa=xpa,
                cb1=cb1,
                cb2=cb2,
                wp=wp,
                w2a=w2a,
                w2b=w2b,
                wenm=wenm,
                ew2b=ew2b,
                ow1=ow1,
                ob1=ob1,
                w2o=w2o,
                id128=id128,
                on16=on16,
            )
        )
    return in_maps


_CACHE = {}


def _get_runner():
    """Build the Bass program once and a cached jitted PJRT executable over
    the 8 cores (shard_map along axis 0 of every input)."""
    if "runner" in _CACHE:
        return _CACHE["runner"]

    import jax
    from jax.sharding import Mesh, NamedSharding, PartitionSpec

    from concourse.bass2jax import (
        _bass_exec_p,
        install_neuronx_cc_hook,
        partition_id_tensor,
        shard_map,
    )

    nc = build_nc()
    _CACHE["nc"] = nc
    install_neuronx_cc_hook()
    partition_name = nc.partition_id_tensor.name if nc.partition_id_tensor else None
    in_names, out_names, out_avals = [], [], []
    for alloc in nc.m.functions[0].allocations:
        if not isinstance(alloc, mybir.MemoryLocationSet):
            continue
        name = alloc.memorylocations[0].name
        if alloc.kind == "ExternalInput":
            if name != partition_name:
                in_names.append(name)
        elif alloc.kind == "ExternalOutput":
            out_names.append(name)
            out_avals.append(
                jax.core.ShapedArray(tuple(alloc.tensor_shape), mybir.dt.np(alloc.dtype))
            )
    all_in_names = tuple(in_names + out_names)
    if partition_name is not None:
        all_in_names = all_in_names + (partition_name,)

    def _body(*args):
        operands = list(args)
        if partition_name is not None:
            operands.append(partition_id_tensor())
        return tuple(
            _bass_exec_p.bind(
                *operands,
                out_avals=tuple(out_avals),
                in_names=all_in_names,
                out_names=tuple(out_names),
                lowering_input_output_aliases=(),
                sim_require_finite=True,
                sim_require_nnan=True,
                nc=nc,
            )
        )

    devices = jax.devices()[:NCORES]
    mesh = Mesh(np.asarray(devices), ("core",))
    nin = len(in_names) + len(out_names)
    fn = jax.jit(
        shard_map(
            _body,
            mesh=mesh,
            in_specs=(PartitionSpec("core"),) * nin,
            out_specs=(PartitionSpec("core"),) * len(out_names),
            check_rep=False,
        ),
        keep_unused=True,
    )
    sharding = NamedSharding(mesh, PartitionSpec("core"))
    runner = (fn, sharding, in_names, out_avals)
    _CACHE["runner"] = runner
    return runner


def kernel(**inputs):
    import jax

    in_maps = make_in_maps(inputs)
    fn, sharding, in_names, out_avals = _get_runner()
    concat_in = [
        np.concatenate([np.asarray(in_maps[c][name]) for c in range(NCORES)], axis=0)
        for name in in_names
    ]
    concat_zeros = [
        np.zeros((NCORES * a.shape[0], *a.shape[1:]), a.dtype) for a in out_avals
    ]
    args = [jax.device_put(a, sharding) for a in (*concat_in, *concat_zeros)]
    (out,) = fn(*args)
    return np.asarray(out).reshape(B).astype(np.float32)

